# revision 5
# baseline (speedup 1.0000x reference)
"""Trainium2 Bass kernel for the BN + 1x1-conv self-attention block.

Reference computation (per batch item, c=256 channels, n=4096 tokens):
  BN(x) over (b,h,w) -> qkv = W_qkv @ xn -> attention -> W_out proj -> +x

Sharding: 8 cores = 4 batch items x 2 query-halves. Each core:
  - holds the full x of its batch item in [c, pos] layout, rotated so its
    2048 query positions come first (attention is permutation-invariant
    in the key axis, so the rotation only needs consistency of K and V)
  - computes partial BN sums over its 2048 positions; a [128,4] AllReduce
    across all 8 cores yields exact global batch statistics
  - folds BN into the QKV conv: W' = W * s_c (per input channel),
    b' = b_qkv + W @ t. The V-channel shift contributes a constant
    per-channel offset to the attention output (softmax weights sum to 1),
    applied post-attention as a per-partition bias.
  - attention is computed in transposed form: S^T[k,q] tiles flow
    scores -> exp -> (A^T V -> out^T[c,q]) with the softmax denominator
    accumulated by a ones-vector matmul; normalization applied at the end.

All heavy matmuls use float32r operands (~1e-4 component error, full PE
rate at free-dim >= 256).
"""
import sys

sys.path.insert(0, "/opt/trn_rl_repo")

import numpy as np
from contextlib import ExitStack

import concourse.bass as bass
import concourse.tile as tile
from concourse import bacc, mybir
from concourse import bass_utils

F32 = mybir.dt.float32
F32R = mybir.dt.float32r
AF = mybir.ActivationFunctionType
ALU = mybir.AluOpType

B, C, H, W = 4, 256, 64, 64
NPOS = H * W          # 4096 positions per item
NQ = NPOS // 2        # 2048 query positions per core
N_CORES = 8
CT = C // 128         # 2 channel partition-tiles
OT = 3 * C // 128     # 6 qkv output tiles
EPS = 1e-5
SCALE = C ** (-0.5)   # 1/16
NTOT = float(B * NPOS)  # BN normalizer 16384


def _build():
    nc = bacc.Bacc("TRN2", target_bir_lowering=False, debug=False)

    x_full = nc.dram_tensor("x_full", [C, NPOS], F32, kind="ExternalInput")
    w_qkv_t = nc.dram_tensor("w_qkv_t", [C, 3 * C], F32, kind="ExternalInput")
    w_out_t = nc.dram_tensor("w_out_t", [C, C], F32, kind="ExternalInput")
    b_qkv = nc.dram_tensor("b_qkv", [3 * C, 1], F32, kind="ExternalInput")
    b_out = nc.dram_tensor("b_out", [C, 1], F32, kind="ExternalInput")
    gamma = nc.dram_tensor("gamma", [C, 1], F32, kind="ExternalInput")
    beta = nc.dram_tensor("beta", [C, 1], F32, kind="ExternalInput")
    out_d = nc.dram_tensor("out", [C, NQ], F32, kind="ExternalOutput")

    with tile.TileContext(nc) as tc:
        with ExitStack() as ctx:
            big = ctx.enter_context(tc.tile_pool(name="big", bufs=1))
            stage = ctx.enter_context(tc.tile_pool(name="stage", bufs=2))
            vec = ctx.enter_context(tc.tile_pool(name="vec", bufs=1))
            expp = ctx.enter_context(tc.tile_pool(name="expp", bufs=4))
            attnp = ctx.enter_context(tc.tile_pool(name="attnp", bufs=4))
            outp = ctx.enter_context(tc.tile_pool(name="outp", bufs=3))
            dram = ctx.enter_context(tc.tile_pool(name="dram", bufs=1, space="DRAM"))
            ps_s = ctx.enter_context(tc.tile_pool(name="ps_s", bufs=2, space="PSUM"))
            ps_av = ctx.enter_context(tc.tile_pool(name="ps_av", bufs=2, space="PSUM"))
            ps_out = ctx.enter_context(tc.tile_pool(name="ps_out", bufs=2, space="PSUM"))
            ps_misc = ctx.enter_context(tc.tile_pool(name="ps_misc", bufs=2, space="PSUM"))

            # ---------------- load weights / vectors ----------------
            w_f32 = []     # W_qkv^T [c', o] fp32, per channel-tile
            for ct in range(CT):
                wt = big.tile([128, 3 * C], F32, tag=f"w_f32_{ct}")
                nc.sync.dma_start(wt[:], w_qkv_t[128 * ct:128 * (ct + 1), :])
                w_f32.append(wt)
            wout_r = []    # W_out^T [c', o] f32r
            for ct in range(CT):
                ws = stage.tile([128, C], F32, tag="wout_stage")
                nc.sync.dma_start(ws[:], w_out_t[128 * ct:128 * (ct + 1), :])
                wr = big.tile([128, C], F32R, tag=f"wout_r_{ct}")
                nc.vector.tensor_copy(wr[:], ws[:])
                wout_r.append(wr)

            bq_col = []    # b_qkv as [128,1] column tiles (6 o-tiles)
            for ot in range(OT):
                t = vec.tile([128, 1], F32, tag=f"bq_col_{ot}")
                nc.sync.dma_start(t[:], b_qkv[128 * ot:128 * (ot + 1), :])
                bq_col.append(t)
            bo_col, ga_col, be_col = [], [], []
            for ct in range(CT):
                t = vec.tile([128, 1], F32, tag=f"bo_{ct}")
                nc.sync.dma_start(t[:], b_out[128 * ct:128 * (ct + 1), :])
                bo_col.append(t)
                t = vec.tile([128, 1], F32, tag=f"ga_{ct}")
                nc.sync.dma_start(t[:], gamma[128 * ct:128 * (ct + 1), :])
                ga_col.append(t)
                t = vec.tile([128, 1], F32, tag=f"be_{ct}")
                nc.sync.dma_start(t[:], beta[128 * ct:128 * (ct + 1), :])
                be_col.append(t)

            eps_col = vec.tile([128, 1], F32, tag="eps_col")
            nc.vector.memset(eps_col[:], EPS)
            ones_col_f = vec.tile([128, 1], F32, tag="ones_col_f")
            nc.vector.memset(ones_col_f[:], 1.0)
            ones_col_r = vec.tile([128, 1], F32R, tag="ones_col_r")
            nc.vector.tensor_copy(ones_col_r[:], ones_col_f[:])
            ones_row_f = vec.tile([1, 128], F32, tag="ones_row_f")
            nc.vector.memset(ones_row_f[:], 1.0)
            ones_row_r = vec.tile([1, 128], F32R, tag="ones_row_r")
            nc.vector.tensor_copy(ones_row_r[:], ones_row_f[:])

            # ---------------- load x, round to f32r ----------------
            xq = []   # x fp32, query half only [128, 2048]
            x_r = []  # x f32r, all positions [128, 4096]
            for ct in range(CT):
                xqt = big.tile([128, NQ], F32, tag=f"xq_{ct}")
                nc.sync.dma_start(xqt[:], x_full[128 * ct:128 * (ct + 1), 0:NQ])
                xq.append(xqt)
                xrt = big.tile([128, NPOS], F32R, tag=f"x_r_{ct}")
                nc.vector.tensor_copy(xrt[:, 0:NQ], xqt[:])
                st = stage.tile([128, NQ], F32, tag="x_stage")
                nc.sync.dma_start(st[:], x_full[128 * ct:128 * (ct + 1), NQ:NPOS])
                nc.vector.tensor_copy(xrt[:, NQ:NPOS], st[:])
                x_r.append(xrt)

            # ---------------- partial BN stats over the 2048-query shard ----
            statp = vec.tile([128, 4], F32, tag="statp")
            for ct in range(CT):
                xg = xq[ct][:].rearrange("p (n f) -> p n f", f=512)
                stats = vec.tile([128, 4, 6], F32, tag="bnstats")
                for i in range(4):
                    nc.vector.bn_stats(out=stats[:, i, :], in_=xg[:, i, :])
                mv = vec.tile([128, 2], F32, tag="bnmv")
                nc.vector.bn_aggr(out=mv[:], in_=stats[:])
                # shard sums: sum = mean*2048 ; sumsq = (var + mean^2)*2048
                nc.scalar.mul(statp[:, 2 * ct:2 * ct + 1], mv[:, 0:1], float(NQ))
                m2 = vec.tile([128, 1], F32, tag="m2")
                nc.vector.tensor_mul(m2[:], mv[:, 0:1], mv[:, 0:1])
                nc.vector.tensor_add(m2[:], m2[:], mv[:, 1:2])
                nc.scalar.mul(statp[:, 2 * ct + 1:2 * ct + 2], m2[:], float(NQ))

            cc_in = dram.tile([128, 4], F32)
            cc_out = dram.tile([128, 4], F32)
            nc.sync.dma_start(cc_in[:], statp[:])
            nc.gpsimd.collective_compute(
                "AllReduce",
                ALU.add,
                replica_groups=[list(range(N_CORES))],
                ins=[cc_in.opt()],
                outs=[cc_out.opt()],
            )
            g_stats = vec.tile([128, 4], F32, tag="g_stats")
            nc.sync.dma_start(g_stats[:], cc_out[:])

            # ---------------- derive s (scale) and t (shift) per channel ----
            s_col, t_col = [], []
            for ct in range(CT):
                mean = vec.tile([128, 1], F32, tag=f"mean_{ct}")
                nc.scalar.mul(mean[:], g_stats[:, 2 * ct:2 * ct + 1], 1.0 / NTOT)
                e2 = vec.tile([128, 1], F32, tag=f"e2_{ct}")
                nc.scalar.mul(e2[:], g_stats[:, 2 * ct + 1:2 * ct + 2], 1.0 / NTOT)
                var = vec.tile([128, 1], F32, tag=f"var_{ct}")
                nc.vector.tensor_mul(var[:], mean[:], mean[:])
                nc.vector.tensor_tensor(out=var[:], in0=e2[:], in1=var[:], op=ALU.subtract)
                sd = vec.tile([128, 1], F32, tag=f"sd_{ct}")
                nc.scalar.activation(sd[:], var[:], AF.Sqrt, bias=eps_col[:])
                rstd = vec.tile([128, 1], F32, tag=f"rstd_{ct}")
                nc.vector.reciprocal(rstd[:], sd[:])
                s = vec.tile([128, 1], F32, tag=f"s_{ct}")
                nc.vector.tensor_mul(s[:], rstd[:], ga_col[ct][:])
                tt = vec.tile([128, 1], F32, tag=f"t_{ct}")
                nc.vector.tensor_mul(tt[:], mean[:], s[:])
                nc.vector.tensor_tensor(out=tt[:], in0=be_col[ct][:], in1=tt[:], op=ALU.subtract)
                s_col.append(s)
                t_col.append(tt)

            # ---------------- fold BN into weights ----------------
            # b'[o] = b_qkv[o] + sum_c W^T[c,o] * t_c   (fp32 matmul, N=1)
            bq_fold = []
            for ot in range(OT):
                pbq = ps_misc.tile([128, 1], F32, tag="misc")
                for ct in range(CT):
                    nc.tensor.matmul(
                        pbq[:],
                        w_f32[ct][:, 128 * ot:128 * (ot + 1)],
                        t_col[ct][:],
                        start=(ct == 0), stop=(ct == CT - 1),
                    )
                bqf = vec.tile([128, 1], F32, tag=f"bqf_{ot}")
                nc.vector.tensor_add(bqf[:], pbq[:], bq_col[ot][:])
                bq_fold.append(bqf)
            # W' = W * s_c (rounded to f32r), per channel-tile row scale
            wqkv_r = []
            for ct in range(CT):
                wr = big.tile([128, 3 * C], F32R, tag=f"wqkv_r_{ct}")
                nc.vector.tensor_scalar_mul(wr[:], w_f32[ct][:], s_col[ct][:])
                wqkv_r.append(wr)

            # ---------------- QKV projections ----------------
            # Q^T and K in [o, pos] layout: psum = W'^T[:,ot] . x_r
            qt_r = [big.tile([128, NQ], F32R, tag=f"qt_{ct}", name=f"qt_{ct}") for ct in range(CT)]
            k_r = [big.tile([128, NPOS], F32R, tag=f"k_{ct}", name=f"k_{ct}") for ct in range(CT)]
            for ot in range(4):  # o-tiles 0,1 -> Q ; 2,3 -> K
                is_q = ot < 2
                npc = 4 if is_q else 8
                for pc in range(npc):
                    ps = ps_s.tile([128, 512], F32, tag="s")
                    for ct in range(CT):
                        nc.tensor.matmul(
                            ps[:],
                            wqkv_r[ct][:, 128 * ot:128 * (ot + 1)],
                            x_r[ct][:, 512 * pc:512 * (pc + 1)],
                            start=(ct == 0), stop=(ct == CT - 1),
                        )
                    if is_q:
                        dest = qt_r[ot][:, 512 * pc:512 * (pc + 1)]
                    else:
                        dest = k_r[ot - 2][:, 512 * pc:512 * (pc + 1)]
                    nc.vector.tensor_scalar_add(dest, ps[:], bq_fold[ot][:])
            # V in [pos, c] layout: psum[p-tile, c] = x_r[:, p-tile]^T . Wv'
            v_r = big.tile([128, 32 * C], F32R, tag="v_r")
            for pt in range(32):
                psv = ps_out.tile([128, C], F32, tag="o")
                for ct in range(CT):
                    nc.tensor.matmul(
                        psv[:],
                        x_r[ct][:, 128 * pt:128 * (pt + 1)],
                        wqkv_r[ct][:, 2 * C:3 * C],
                        start=(ct == 0), stop=(ct == CT - 1),
                    )
                nc.vector.tensor_copy(v_r[:, C * pt:C * (pt + 1)], psv[:])

            # ---------------- attention, streamed per 512-query chunk -------
            for qc in range(NQ // 512):
                qs = slice(512 * qc, 512 * (qc + 1))
                av = [ps_av.tile([128, 512], F32, tag="av", name=f"av_q{qc}_{_}") for _ in range(CT)]
                dn = ps_misc.tile([1, 512], F32, tag="misc")
                for kt in range(32):
                    ss = ps_s.tile([128, 512], F32, tag="s")
                    for ct in range(CT):
                        nc.tensor.matmul(
                            ss[:],
                            k_r[ct][:, 128 * kt:128 * (kt + 1)],
                            qt_r[ct][:, qs],
                            start=(ct == 0), stop=(ct == CT - 1),
                        )
                    ex = expp.tile([128, 512], F32R, tag="ex")
                    nc.scalar.activation(ex[:], ss[:], AF.Exp, scale=SCALE)
                    for ct in range(CT):
                        nc.tensor.matmul(
                            av[ct][:],
                            v_r[:, C * kt + 128 * ct:C * kt + 128 * (ct + 1)],
                            ex[:],
                            start=(kt == 0), stop=(kt == 31),
                        )
                    nc.tensor.matmul(
                        dn[:], ones_col_r[:], ex[:],
                        start=(kt == 0), stop=(kt == 31),
                    )
                # normalize: attn^T[c,q] = av * (1/dn) + bv'
                dn_sb = vec.tile([1, 512], F32, tag="dn_sb")
                nc.vector.tensor_copy(dn_sb[:], dn[:])
                rec = vec.tile([1, 512], F32, tag="rec")
                nc.vector.reciprocal(rec[:], dn_sb[:])
                rec_r = vec.tile([1, 512], F32R, tag="rec_r")
                nc.vector.tensor_copy(rec_r[:], rec[:])
                bc = ps_misc.tile([128, 512], F32, tag="misc")
                nc.tensor.matmul(bc[:], ones_row_r[:], rec_r[:], start=True, stop=True)
                bc_sb = attnp.tile([128, 512], F32, tag="bc_sb")
                nc.scalar.mul(bc_sb[:], bc[:], 1.0)
                at_sb = []
                for ct in range(CT):
                    at = attnp.tile([128, 512], F32R, tag=f"at_{ct}")
                    nc.vector.tensor_tensor(out=at[:], in0=av[ct][:], in1=bc_sb[:], op=ALU.mult)
                    nc.vector.tensor_scalar_add(at[:], at[:], bq_fold[4 + ct][:])
                    at_sb.append(at)
                # output projection + bias + residual
                for ot in range(CT):
                    po = ps_out.tile([128, 512], F32, tag="o")
                    for ct in range(CT):
                        nc.tensor.matmul(
                            po[:],
                            wout_r[ct][:, 128 * ot:128 * (ot + 1)],
                            at_sb[ct][:],
                            start=(ct == 0), stop=(ct == CT - 1),
                        )
                    fin = outp.tile([128, 512], F32, tag="fin")
                    nc.vector.tensor_scalar_add(fin[:], po[:], bo_col[ot][:])
                    nc.vector.tensor_tensor(out=fin[:], in0=fin[:], in1=xq[ot][:, qs], op=ALU.add)
                    nc.sync.dma_start(out_d[128 * ot:128 * (ot + 1), qs], fin[:])

    nc.finalize()
    return nc


_NC_CACHE = None


def _get_nc():
    global _NC_CACHE
    if _NC_CACHE is None:
        _NC_CACHE = _build()
    return _NC_CACHE


def kernel(x, W_qkv, b_qkv, W_out, b_out, gamma, beta):
    x = np.asarray(x, dtype=np.float32)
    W_qkv = np.asarray(W_qkv, dtype=np.float32)
    b_qkv = np.asarray(b_qkv, dtype=np.float32)
    W_out = np.asarray(W_out, dtype=np.float32)
    b_out = np.asarray(b_out, dtype=np.float32)
    gamma = np.asarray(gamma, dtype=np.float32)
    beta = np.asarray(beta, dtype=np.float32)

    nc = _get_nc()

    w_qkv_t = np.ascontiguousarray(W_qkv.T)          # [256, 768]
    w_out_t = np.ascontiguousarray(W_out.T)          # [256, 256]
    bq2 = b_qkv.reshape(3 * C, 1)
    bo2 = b_out.reshape(C, 1)
    ga2 = gamma.reshape(C, 1)
    be2 = beta.reshape(C, 1)

    xf = x.reshape(B, C, NPOS)
    in_maps = []
    for core in range(N_CORES):
        item, half = divmod(core, 2)
        xi = xf[item]
        if half == 0:
            xr = xi
        else:
            xr = np.concatenate([xi[:, NQ:], xi[:, :NQ]], axis=1)
        in_maps.append({
            "x_full": np.ascontiguousarray(xr),
            "w_qkv_t": w_qkv_t,
            "w_out_t": w_out_t,
            "b_qkv": bq2,
            "b_out": bo2,
            "gamma": ga2,
            "beta": be2,
        })

    res = bass_utils.run_bass_kernel_spmd(nc, in_maps, core_ids=list(range(N_CORES)))

    out = np.empty((B, C, NPOS), dtype=np.float32)
    for core in range(N_CORES):
        item, half = divmod(core, 2)
        out[item][:, NQ * half:NQ * (half + 1)] = res.results[core]["out"]
    return out.reshape(B, C, H, W)
